# revision 1
# baseline (speedup 1.0000x reference)
"""Trainium2 Bass kernel for multi-head attention (dense_transformer).

Full module: qkv = x @ W_qkv + b_qkv; multi-head attention (16 heads, d=64,
N=4096); out = attn @ W_proj + b_proj.

Sharding: tensor-parallel over heads — 2 heads per core on 8 cores. Each core
receives full x (pre-transposed on host to [C, N]) plus its head-slices of the
weights, computes its heads' attention and a partial output projection
[N, C]; the host sums the 8 partials and adds b_proj.

Per-core dataflow (matmul operands in fp16 — 16-bit streaming is ~5x
faster than fp32/fp32r on the PE moving-operand path; PSUM accumulates fp32):
  A) Q^T,K^T [128, 4096] = W^T @ x^T accumulated over C chunks (PSUM), bias
     added on eviction.  V^T likewise, then PE-transposed to V natural
     [tok, d] stored with a constant ones column per head: [V_h | 1].
  B) per (q-chunk 512, k-chunk 128): S^T [128, 1024] for both heads packed
     side by side (row-tiled K=64 matmuls), ACT exp (scale=1/8) from PSUM to
     SBUF fp16, then AV matmuls lhsT=[V_h|1] accumulate out^T[65, 512] whose
     row 64 is the softmax denominator.  Normalize with DVE reciprocal +
     GPSIMD partition broadcast + DVE mul, add b_v.
  C) partial proj: out[tok,  C] = attn_out^T.T @ W_proj_slice, evicted by DVE
     and DMAd out.
"""

import numpy as np
from contextlib import ExitStack

NUM_CORES = 8
DIM = 1024
NUM_HEADS = 16
HDIM = 64
N = 4096
HPC = NUM_HEADS // NUM_CORES   # heads per core = 2
DPC = HPC * HDIM               # head dims per core = 128

_NC_CACHE = {}


def build_nc(reps=1, trace_sim=False):
    if (reps, trace_sim) in _NC_CACHE:
        return _NC_CACHE[(reps, trace_sim)]

    import concourse.bass as bass
    import concourse.mybir as mybir
    import concourse.tile as tile
    from concourse import bacc
    from concourse.masks import make_identity

    f32 = mybir.dt.float32
    fp16 = mybir.dt.float16
    AF = mybir.ActivationFunctionType
    ts = bass.ts

    nc = bacc.Bacc(trn_type="TRN2", target_bir_lowering=False, debug=False)
    xT = nc.dram_tensor("xT", [DIM, N], fp16, kind="ExternalInput").ap()
    wq = nc.dram_tensor("wq", [DIM, DPC], fp16, kind="ExternalInput").ap()
    wk = nc.dram_tensor("wk", [DIM, DPC], fp16, kind="ExternalInput").ap()
    wv = nc.dram_tensor("wv", [DIM, DPC], fp16, kind="ExternalInput").ap()
    wp = nc.dram_tensor("wp", [DPC, DIM], fp16, kind="ExternalInput").ap()
    bq = nc.dram_tensor("bq", [DPC, 1], f32, kind="ExternalInput").ap()
    bk = nc.dram_tensor("bk", [DPC, 1], f32, kind="ExternalInput").ap()
    bv = nc.dram_tensor("bv", [DPC, 1], f32, kind="ExternalInput").ap()
    ones = nc.dram_tensor("ones", [1, 1], fp16, kind="ExternalInput").ap()
    out = nc.dram_tensor("out", [N, DIM], f32, kind="ExternalOutput").ap()

    with tile.TileContext(nc, trace_sim=trace_sim) as tc, ExitStack() as ctx:
        singles = ctx.enter_context(tc.tile_pool(name="singles", bufs=1))
        psum = ctx.enter_context(tc.tile_pool(name="ps", bufs=2, space="PSUM"))
        xpool = ctx.enter_context(tc.tile_pool(name="xp", bufs=2))
        work = ctx.enter_context(tc.tile_pool(name="work", bufs=2))
        ppool = ctx.enter_context(tc.tile_pool(name="pp", bufs=3))
        opool = ctx.enter_context(tc.tile_pool(name="op", bufs=3))

        ident = singles.tile([128, 128], f32, tag="ident")
        make_identity(nc, ident)

        wq_sb = singles.tile([128, 8, DPC], fp16, tag="wq")
        wk_sb = singles.tile([128, 8, DPC], fp16, tag="wk")
        wv_sb = singles.tile([128, 8, DPC], fp16, tag="wv")
        nc.sync.dma_start(out=wq_sb, in_=wq.rearrange("(c p) m -> p c m", p=128))
        nc.sync.dma_start(out=wk_sb, in_=wk.rearrange("(c p) m -> p c m", p=128))
        nc.sync.dma_start(out=wv_sb, in_=wv.rearrange("(c p) m -> p c m", p=128))
        wp_sb = singles.tile([64, HPC, DIM], fp16, tag="wp")
        nc.sync.dma_start(out=wp_sb, in_=wp.rearrange("(h d) c -> d h c", d=64))
        bq_sb = singles.tile([DPC, 1], f32, tag="bq")
        bk_sb = singles.tile([DPC, 1], f32, tag="bk")
        nc.sync.dma_start(out=bq_sb, in_=bq)
        nc.sync.dma_start(out=bk_sb, in_=bk)
        bv_sb = singles.tile([64, HPC, 1], f32, tag="bv")
        nc.sync.dma_start(out=bv_sb, in_=bv.rearrange("(h d) x -> d h x", d=64))

        qT = singles.tile([128, N], fp16, tag="qT")
        kT = singles.tile([128, N], fp16, tag="kT")
        aoT0 = singles.tile([64, N], fp16, tag="aoT0")
        aoT1 = singles.tile([64, N], fp16, tag="aoT1")
        # V natural layout + ones column per head: [.., t, 0:64]=V_h0,
        # [.., t, 64]=1, [.., t, 65:129]=V_h1, [.., t, 129]=1
        v_nat = singles.tile([128, 32, 130], fp16, tag="vnat")
        # ones columns loaded via broadcast DMA
        nc.sync.dma_start(out=v_nat[:, :, 64:65], in_=ones.to_broadcast((128, 32, 1)))
        nc.sync.dma_start(out=v_nat[:, :, 129:130], in_=ones.to_broadcast((128, 32, 1)))

        for _rep in range(reps):
            # ---------------- Phase A: QKV projection ----------------
            for qt in range(4):
                xt = [xpool.tile([128, 1024], fp16, tag=f"x{c}", name=f"x{c}") for c in range(8)]
                for c in range(8):
                    nc.sync.dma_start(out=xt[c], in_=xT[ts(c, 128), ts(qt, 1024)])
                for nl in range(2):
                    n = qt * 2 + nl
                    # K and V first: attention waits on full K/V, while Q
                    # chunks are consumed per q-tile
                    acc = psum.tile([128, 512], f32, tag="pj", name="kacc", bufs=2)
                    for c in range(8):
                        nc.tensor.matmul(
                            acc, wk_sb[:, c, :], xt[c][:, ts(nl, 512)],
                            start=(c == 0), stop=(c == 7),
                        )
                    nc.vector.tensor_scalar_add(kT[:, ts(n, 512)], acc, bk_sb)
                    vacc = psum.tile([128, 512], f32, tag="pj", name="vacc", bufs=2)
                    for c in range(8):
                        nc.tensor.matmul(
                            vacc, wv_sb[:, c, :], xt[c][:, ts(nl, 512)],
                            start=(c == 0), stop=(c == 7),
                        )
                    vst = work.tile([128, 512], f32, tag="vst")
                    nc.vector.tensor_copy(vst, vacc)
                    tpb = psum.tile([128, 512], f32, tag="big", name="tpb", bufs=2)
                    for tl in range(4):
                        nc.tensor.transpose(
                            tpb[:, ts(tl, 128)], vst[:, ts(tl, 128)], ident)
                    nc.vector.tensor_copy(
                        out=v_nat[:, ts(n, 4), 0:130]
                        .rearrange("p t (g d) -> p t g d", d=65)[:, :, :, 0:64],
                        in_=tpb.rearrange("p (t g d) -> p t g d", g=2, d=64),
                    )
                    qacc = psum.tile([128, 512], f32, tag="big", name="qacc", bufs=2)
                    for c in range(8):
                        nc.tensor.matmul(
                            qacc, wq_sb[:, c, :], xt[c][:, ts(nl, 512)],
                            start=(c == 0), stop=(c == 7),
                        )
                    nc.vector.tensor_scalar_add(qT[:, ts(n, 512)], qacc, bq_sb)

            # ---------------- Phase B: attention + lagged projection ------
            def emit_proj_chunk(t, j):
                pp = psum.tile([128, 512], f32, tag="pj", name="pp", bufs=2)
                nc.tensor.matmul(
                    pp, aoT0[:, ts(t, 128)], wp_sb[:, 0, ts(j, 512)],
                    start=True, stop=False,
                )
                nc.tensor.matmul(
                    pp, aoT1[:, ts(t, 128)], wp_sb[:, 1, ts(j, 512)],
                    start=False, stop=True,
                )
                ot = opool.tile([128, 512], f32, tag="ot")
                nc.vector.tensor_copy(ot, pp)
                nc.sync.dma_start(out=out[ts(t, 128), ts(j, 512)], in_=ot)

            for qi in range(8):
                # proj tasks for the previous q-chunk, spread across this
                # q-chunk's ki loop so they fill PE slack without stalling
                # the score/exp stream
                proj_tasks = (
                    [((qi - 1) * 4 + tl, j) for tl in range(4) for j in range(2)]
                    if qi >= 1 else []
                )
                av = [
                    psum.tile([65, 512], f32, tag="av0", name="av0", bufs=1),
                    psum.tile([65, 512], f32, tag="av1", name="av1", bufs=1),
                ]
                # software-pipelined: emit scores/exp one step ahead of AV
                p_tiles = {}
                for ki in range(33):
                    if ki < 32:
                        s = psum.tile([128, 1024], f32, tag="big", name="s", bufs=2)
                        nc.tensor.matmul(
                            s[:, 0:512], kT[0:64, ts(ki, 128)], qT[0:64, ts(qi, 512)],
                            start=True, stop=True,
                        )
                        nc.tensor.matmul(
                            s[:, 512:1024], kT[64:128, ts(ki, 128)],
                            qT[64:128, ts(qi, 512)],
                            start=True, stop=True,
                        )
                        p = ppool.tile([128, 1024], fp16, tag="p")
                        nc.scalar.activation(p, s, AF.Exp, scale=0.125)
                        p_tiles[ki] = p
                    if ki >= 1:
                        kj = ki - 1
                        p = p_tiles.pop(kj)
                        nc.tensor.matmul(
                            av[0], v_nat[:, kj, 0:65], p[:, 0:512],
                            start=(kj == 0), stop=(kj == 31),
                        )
                        nc.tensor.matmul(
                            av[1], v_nat[:, kj, 65:130], p[:, 512:1024],
                            start=(kj == 0), stop=(kj == 31),
                        )
                    if ki % 4 == 2 and proj_tasks:
                        emit_proj_chunk(*proj_tasks.pop(0))
                for h, (acc, aoT) in enumerate(((av[0], aoT0), (av[1], aoT1))):
                    # single fast copy releases the PSUM accumulator slot so
                    # the next q-chunk's AV matmuls aren't gated on the whole
                    # normalize chain; normalize runs from the SBUF staging
                    avs = work.tile([65, 512], f32, tag="avs", name="avs",
                                    bufs=4)
                    nc.vector.tensor_copy(avs, acc)
                    recip = work.tile([1, 512], f32, tag="recip", name="recip")
                    nc.vector.reciprocal(recip, avs[64:65, :])
                    bc = work.tile([64, 512], f32, tag="bc", name="bc")
                    nc.gpsimd.partition_broadcast(bc, recip)
                    nc.vector.tensor_mul(aoT[:, ts(qi, 512)], avs[0:64, :], bc)
                    nc.vector.tensor_scalar_add(
                        aoT[:, ts(qi, 512)], aoT[:, ts(qi, 512)], bv_sb[:, h, :]
                    )
            # tail: projection of the final q-chunk
            for tl in range(4):
                for j in range(2):
                    emit_proj_chunk(7 * 4 + tl, j)

    nc.compile()
    _NC_CACHE[(reps, trace_sim)] = nc
    return nc


def make_in_maps(x, W_qkv, b_qkv, W_proj):
    x2 = np.asarray(x, dtype=np.float32).reshape(N, DIM)
    xTv = np.ascontiguousarray(x2.T.astype(np.float16))
    W_qkv = np.asarray(W_qkv, dtype=np.float32)
    W16 = W_qkv.astype(np.float16)
    b_qkv = np.asarray(b_qkv, dtype=np.float32)
    Wp16 = np.asarray(W_proj, dtype=np.float32).astype(np.float16)
    maps = []
    for m in range(NUM_CORES):
        h0 = m * DPC
        maps.append({
            "xT": xTv,
            "wq": np.ascontiguousarray(W16[:, h0:h0 + DPC]),
            "wk": np.ascontiguousarray(W16[:, DIM + h0:DIM + h0 + DPC]),
            "wv": np.ascontiguousarray(W16[:, 2 * DIM + h0:2 * DIM + h0 + DPC]),
            "wp": np.ascontiguousarray(Wp16[h0:h0 + DPC, :]),
            "bq": np.ascontiguousarray(b_qkv[h0:h0 + DPC].reshape(DPC, 1)),
            "bk": np.ascontiguousarray(
                b_qkv[DIM + h0:DIM + h0 + DPC].reshape(DPC, 1)),
            "bv": np.ascontiguousarray(
                b_qkv[2 * DIM + h0:2 * DIM + h0 + DPC].reshape(DPC, 1)),
            "ones": np.ones((1, 1), dtype=np.float16),
        })
    return maps


def kernel(x, W_qkv, b_qkv, W_proj, b_proj, _reps=1):
    from concourse.bass_utils import run_bass_kernel_spmd

    nc = build_nc(_reps)
    maps = make_in_maps(x, W_qkv, b_qkv, W_proj)
    res = run_bass_kernel_spmd(nc, maps, list(range(NUM_CORES)))
    partial = np.stack([r["out"] for r in res.results], axis=0)
    total = partial.sum(axis=0, dtype=np.float32)
    total = total + np.asarray(b_proj, dtype=np.float32)[None, :]
    return total.reshape(1, N, DIM).astype(np.float32)

